# revision 17
# baseline (speedup 1.0000x reference)
"""Trainium2 Bass kernel v2: DGCNN forward (4-layer GCN + Conv1d readout) on 8 cores.

Math (same as v1): with A = D^-1/2 (Adj + I) D^-1/2,
    out = A(x M1 + A(x M2 + A(x M3 + A(x M4)))) + bias-table
All 4 aggregation passes are width-16 gathers + segment sums.

v2 structural changes vs v1 (v1 = kernel_baseline.py):
  - Half-major global node layout: position = h1: k*4096 + b*128 + j (blocks
    0..31), h2: 32768 + k*2176 + (b-32)*128 + j (blocks 32..48). The half-1
    exchange output IS table rows [0, 32768) = gather window W1, so next-pass
    W1 gathers only wait on the half-1 collective (emitted mid-pass) and the
    pass tail overlaps the half-2 exchange.
  - Two gather windows W1=[0,32768), W2=[17408,50176); flex edges (src in the
    overlap) prefer W1 so the early-firing W1 phase carries ~50% of the work.
  - Pass-0 table (T3 = dinv*(x@M4)) and stage are host-precomputed inputs, so
    pass-0 gathers start right after the idx tensor loads (no startup
    exchange).
  - 48 gather units per pass (16 groups x {W1a, W1b, W2}) rotating across
    the 4 SWDGE queues = 4 Q7 desc-gen core pairs; deep gt pools keep all 4
    pairs busy (desc-gen is the machine bottleneck at ~8ns/desc/pair).
  - Iterated (deg, f2) node dealing cuts ELL padding 1.48x -> 1.27x; bf16
    message tables halve the exchange payload; h1-group units are emitted
    first so the half-1 collective dispatches at ~60% of the pass stream.
  - Per-block accumulator tiles decouple W1-phase and W2-phase consumption.
"""

import dataclasses
import numpy as np

import concourse.bass as bass
import concourse.bacc as bacc
import concourse.tile as tile
from concourse import mybir
from concourse.bass_utils import run_bass_kernel_spmd

F32 = mybir.dt.float32
BF16 = mybir.dt.bfloat16
I16 = mybir.dt.int16
AF = mybir.ActivationFunctionType


@dataclasses.dataclass(frozen=True)
class Cfg:
    N: int = 50000
    F: int = 64
    NCORES: int = 8
    P: int = 128
    NBLK: int = 49
    NBLK_H1: int = 32          # h1 blocks; h1 rows = 8*32*128 = 32768 = W1
    NGRP_H1: int = 10
    NGRP_H2: int = 6
    NQ: int = 4
    SINGLE_PACKET: bool = False
    GT_BUFS: int = 8
    W1_PRE: int = 8            # W1 groups emitted before the h1 W2 block

    @property
    def PER(self):
        return self.NBLK * self.P          # 6272 rows per core

    @property
    def NPAD(self):
        return self.NCORES * self.PER      # 50176

    @property
    def H1ROWS(self):
        return self.NCORES * self.NBLK_H1 * self.P   # 32768

    @property
    def NGRP(self):
        return self.NGRP_H1 + self.NGRP_H2

    # window [start, end) in global positions
    @property
    def WIN(self):
        return ((0, 32768), (self.NPAD - 32768, self.NPAD))


CFG = Cfg()

LAST_RESULTS = None


# --------------------------------------------------------------------------
# host preprocessing
# --------------------------------------------------------------------------

def _global_pos(cfg, k, b, j):
    b = np.asarray(b)
    h1 = b < cfg.NBLK_H1
    r1 = cfg.NBLK_H1 * cfg.P
    r2 = (cfg.NBLK - cfg.NBLK_H1) * cfg.P
    return np.where(
        h1,
        k * r1 + b * cfg.P + j,
        cfg.H1ROWS + k * r2 + (b - cfg.NBLK_H1) * cfg.P + j,
    )


def _host_prep(inputs, cfg: Cfg):
    x = np.asarray(inputs["x"], np.float32)
    ei = np.asarray(inputs["edge_index"]).astype(np.int64)
    W = [np.asarray(inputs[f"W{i}"], np.float64) for i in range(4)]
    b = [np.asarray(inputs[f"b{i}"], np.float64) for i in range(4)]
    conv_w = np.asarray(inputs["conv_w"], np.float64)
    conv_b = np.asarray(inputs["conv_b"], np.float64)

    n = x.shape[0]
    assert n == cfg.N and x.shape[1] == cfg.F
    P, PER, NPAD, NBLK, NC = cfg.P, cfg.PER, cfg.NPAD, cfg.NBLK, cfg.NCORES

    src_l = np.concatenate([ei[0], np.arange(n, dtype=np.int64)])
    dst_l = np.concatenate([ei[1], np.arange(n, dtype=np.int64)])
    deg = np.bincount(dst_l, minlength=n).astype(np.float64)
    dinv = 1.0 / np.sqrt(np.maximum(deg, 1.0))

    # ---- weight-derived small matrices ----
    Cw = [conv_w[:, 0:64], conv_w[:, 64:128], conv_w[:, 128:192], conv_w[:, 192:193]]
    M1 = W[0] @ Cw[0].T
    M2 = W[0] @ W[1] @ Cw[1].T
    M3 = W[0] @ W[1] @ W[2] @ Cw[2].T
    M4 = W[0] @ W[1] @ W[2] @ W[3] @ Cw[3].T
    c0 = b[0] @ Cw[0].T + b[1] @ Cw[1].T + b[2] @ Cw[2].T + b[3] @ Cw[3].T + conv_b
    c1 = (b[0] @ W[1]) @ Cw[1].T + (b[1] @ W[2]) @ Cw[2].T + (b[2] @ W[3]) @ Cw[3].T
    c2 = (b[0] @ W[1] @ W[2]) @ Cw[2].T + (b[1] @ W[2] @ W[3]) @ Cw[3].T
    c3 = (b[0] @ W[1] @ W[2] @ W[3]) @ Cw[3].T

    def aggv(v):
        o = np.zeros(n)
        np.add.at(o, dst_l, (v * dinv)[src_l])
        return o * dinv

    v1 = aggv(np.ones(n))
    v2 = aggv(v1)
    v3 = aggv(v2)
    bias = (np.outer(np.ones(n), c0) + np.outer(v1, c1)
            + np.outer(v2, c2) + np.outer(v3, c3))  # [n, 16]

    # ---- permutation: iterated (deg, f2) dealing into (class, core, j) ----
    # Sorting nodes by (deg desc, f2 asc) before dealing equalizes both the
    # block degree max AND the forced-W2 max within each block, cutting ELL
    # padding ~1.48x -> ~1.25x.  f2 depends on src positions, so iterate.
    ndeg_n = (deg - 1).astype(np.int64)     # non-self in-degree

    def build_perm(order):
        order_p = np.concatenate([order, np.full(NPAD - n, -1, np.int64)])
        deg_p = np.zeros(NPAD)
        real_rank = order_p >= 0
        deg_p[real_rank] = deg[order_p[real_rank]] - 1.0
        cls_of_rank = np.arange(NPAD) // P // NC
        mTc = np.zeros(NBLK)
        np.maximum.at(mTc, cls_of_rank, deg_p)
        # group classes; h1 groups draw from classes 0..31, h2 from 32..48
        cap = ([4, 4] + [3] * (cfg.NGRP_H1 - 2)) + ([3] * (cfg.NGRP_H2 - 1) + [2])
        assert sum(cap[:cfg.NGRP_H1]) == cfg.NBLK_H1 and sum(cap) == NBLK
        groups_c = [[] for _ in range(cfg.NGRP)]
        gsum = np.zeros(cfg.NGRP)
        for bq in np.argsort(-mTc, kind="stable"):
            lo, hi = (0, cfg.NGRP_H1) if bq < cfg.NBLK_H1 else (cfg.NGRP_H1, cfg.NGRP)
            cand = sorted(range(lo, hi),
                          key=lambda q: (len(groups_c[q]) >= cap[q], gsum[q], q))
            q = cand[0]
            groups_c[q].append(int(bq))
            gsum[q] += mTc[bq]
        order_cls = [c for q in range(cfg.NGRP) for c in groups_c[q]]
        renum = np.zeros(NBLK, np.int64)
        for newid, c in enumerate(order_cls):
            renum[c] = newid
        groups = []
        pos = 0
        for q in range(cfg.NGRP):
            groups.append(list(range(pos, pos + len(groups_c[q]))))
            pos += len(groups_c[q])
        g = np.arange(NPAD) // P
        j = np.arange(NPAD) % P
        npos_of_rank = _global_pos(cfg, g % NC, renum[g // NC], j)
        return order_p, renum, groups, npos_of_rank

    order = np.argsort(-deg, kind="stable")
    for _ in range(6):
        order_p, renum, groups, npos_of_rank = build_perm(order)
        o2p = np.full(n, -1, np.int64)
        m_ = order_p >= 0
        o2p[order_p[m_]] = npos_of_rank[m_]
        s_pos = o2p[ei[0]]
        f2n = np.bincount(ei[1][s_pos >= 32768], minlength=n)
        order = np.lexsort((f2n, -ndeg_n))
    order_p, renum, groups, npos_of_rank = build_perm(order)

    # force dummy (pad) nodes at positions 32767 (in W1&W3) and NPAD-1 (in W2)
    pos2rank = np.empty(NPAD, np.int64)
    pos2rank[npos_of_rank] = np.arange(NPAD)
    for dpos in (32767, NPAD - 1):
        r_t = pos2rank[dpos]
        if order_p[r_t] >= 0:
            # swap with some rank that is a pad
            pads = np.nonzero(order_p < 0)[0]
            r_p = pads[-1]
            order_p[r_t], order_p[r_p] = order_p[r_p], order_p[r_t]
    Z1, Z2 = 32767, NPAD - 1      # zero rows: Z1 in W1/W3 range, Z2 in W2
    assert order_p[pos2rank[Z1]] < 0 and order_p[pos2rank[Z2]] < 0

    pos2old = np.full(NPAD, -1, np.int64)
    pos2old[npos_of_rank] = order_p
    old2new = np.full(n, -1, np.int64)
    rmask = pos2old >= 0
    old2new[pos2old[rmask]] = np.nonzero(rmask)[0]
    assert (old2new >= 0).all()

    # ---- per-block 2-window split (flex overlap prefers W1) ----
    (w1s, w1e), (w2s, w2e) = cfg.WIN
    s_new = old2new[ei[0]]
    d_new = old2new[ei[1]]

    # dst position -> (core, block, j)
    def pos_to_kbj(pos):
        h1 = pos < cfg.H1ROWS
        r1 = cfg.NBLK_H1 * P
        r2 = (NBLK - cfg.NBLK_H1) * P
        k = np.where(h1, pos // r1, (pos - cfg.H1ROWS) // r2)
        rem = np.where(h1, pos - k * r1, pos - cfg.H1ROWS - k * r2)
        blk = np.where(h1, rem // P, cfg.NBLK_H1 + rem // P)
        jj = rem % P
        return k, blk, jj

    e_k, e_b, e_j = pos_to_kbj(d_new)
    # edge class: 0=F1(W1 only) 1=flex(W1 or W2) 2=F2(W2 only)
    ecls = np.where(s_new < w2s, 0, np.where(s_new < w1e, 1, 2))

    cnt = np.zeros((3, NPAD), np.int64)
    for c in range(3):
        cnt[c] = np.bincount(d_new[ecls == c], minlength=NPAD)
    f1, mfx, f2 = cnt
    degp = f1 + mfx + f2

    blk_of_pos = pos_to_kbj(np.arange(NPAD))[1]
    S1 = np.zeros(NBLK, np.int64)
    S2 = np.zeros(NBLK, np.int64)
    for bq in range(NBLK):
        sel = blk_of_pos == bq
        mT = int(degp[sel].max())
        mf1 = int(f1[sel].max())
        mf2 = int(f2[sel].max())
        tot = max(mT, mf1 + mf2)
        S2[bq] = mf2            # W2 minimal -> W1 (early phase) maximal
        S1[bq] = tot - mf2
    S3 = np.zeros(NBLK, np.int64)   # no third window

    S1p = S1[blk_of_pos]
    S2p = S2[blk_of_pos]
    a_j = np.minimum(mfx, S1p - f1)        # flex edges going to W1
    n1 = f1 + a_j
    n2 = degp - n1
    assert (n1 <= S1p).all() and (n2 <= S2p).all()

    # ---- per-edge window + slot assignment ----
    eo = np.argsort(d_new, kind="stable")
    d_s = d_new[eo]
    s_s = s_new[eo]
    c_s = ecls[eo]
    starts = np.searchsorted(d_s, np.arange(NPAD + 1))

    def rank_within(mask):
        cm = np.concatenate([[0], np.cumsum(mask)])
        return cm[:-1] - cm[starts[d_s]]

    is_fx = c_s == 1
    rfx = rank_within(is_fx)
    goW1 = (c_s == 0) | (is_fx & (rfx < a_j[d_s]))
    win_of = np.where(goW1, 0, 1)
    slot = np.empty(len(d_s), np.int64)
    for w, m in enumerate((goW1, ~goW1)):
        slot[m] = rank_within(m)[m]
    nw = np.stack([n1, n2])
    assert (slot < nw[win_of, d_s]).all()

    # ---- units ----
    # phase A: for g: W1a(g), W1b(g)  (split group blocks ~in half)
    # phase B: for g: W2(g)           (h1 groups first == natural order)
    Sw = [S1, S2]
    units = []          # dicts: w, g, blocks(list), nslots, colbase
    blk_unit = {}       # (w, b) -> (unit_idx, slot_offset_within_unit)
    colbase = 0

    def add_unit(w, gq, blks):
        nonlocal colbase
        ns = int(sum(Sw[w][bb] for bb in blks))
        if ns == 0:
            return
        u = dict(w=w, g=gq, blocks=list(blks), nslots=ns, colbase=colbase)
        off = 0
        for bb in blks:
            blk_unit[(w, bb)] = (len(units), off)
            off += int(Sw[w][bb])
        colbase += ns * P // 16
        units.append(u)

    # h1 groups' W1 AND W2 units first: their epilogues complete ~60% into
    # the stream, so the h1 collective dispatches early and its latency
    # overlaps the h2-group units that follow.
    def add_w1(gq):
        bl = groups[gq]
        h = (len(bl) + 1) // 2
        add_unit(0, gq, bl[:h])
        add_unit(0, gq, bl[h:])

    w1_next = 0
    while w1_next < cfg.W1_PRE:
        add_w1(w1_next)
        w1_next += 1
    for gq in range(cfg.NGRP_H1):
        while w1_next <= gq:          # invariant: W1(g) precedes W2(g)
            add_w1(w1_next)
            w1_next += 1
        add_unit(1, gq, groups[gq])
        if gq % 2 == 0 and w1_next < cfg.NGRP_H1:
            add_w1(w1_next)
            w1_next += 1
    while w1_next < cfg.NGRP:
        add_w1(w1_next)
        w1_next += 1
    for gq in range(cfg.NGRP_H1, cfg.NGRP):
        add_unit(1, gq, groups[gq])
    idxcols = colbase

    # ---- idx tensor build ----
    WSTART = np.array([w1s, w2s], np.int64)
    ZPAD = np.array([Z1 - w1s, Z2 - w2s], np.int64)
    idx_np = np.empty((NC, 128, idxcols), np.int16)
    for u in units:
        c0_, c1_ = u["colbase"], u["colbase"] + u["nslots"] * P // 16
        idx_np[:, :, c0_:c1_] = np.int16(ZPAD[u["w"]])

    e_uid = np.empty(len(d_s), np.int64)
    e_uoff = np.empty(len(d_s), np.int64)
    for (w, bb), (ui, off) in blk_unit.items():
        m = (win_of == w) & (e_b[eo] == bb)
        e_uid[m] = ui
        e_uoff[m] = off
    ub = np.array([u["colbase"] for u in units], np.int64)
    posn = (e_uoff + slot) * P + e_j[eo]
    col = ub[e_uid] + posn // 16
    row = posn % 16
    val = (s_s - WSTART[win_of]).astype(np.int16)
    assert (val >= 0).all()
    ek = e_k[eo]
    for k in range(NC):
        mk = ek == k
        for r in range(8):
            idx_np[k, row[mk] + 16 * r, col[mk]] = val[mk]

    # ---- consumption plan (host-side schedule metadata) ----
    # For each unit index: list of (block, src_unit, lo, hi, mode) reduces
    # after this unit, mode: 0 = write acc, 1 = add to acc.  Plus epilogues.
    first_w = np.where(S1 > 0, 0, 1)     # first window with slots
    consume = [dict(reduces=[], epilogue=[], exch=None) for _ in units]
    for gq in range(cfg.NGRP):
        bl = groups[gq]
        # phase A consumption: after the last W1 unit of this group
        w1_units = [blk_unit[(0, bb)][0] for bb in bl if (0, bb) in blk_unit]
        if w1_units:
            ui = max(w1_units)
            for bb in bl:
                if (0, bb) in blk_unit:
                    uu, off = blk_unit[(0, bb)]
                    consume[ui]["reduces"].append(
                        (bb, uu, off, off + int(S1[bb]), 0))
            for bb in bl:
                if int(S2[bb]) == 0:
                    consume[ui]["epilogue"].append(bb)   # W1-only blocks
        # phase B: after W2(g)
        b_units = [blk_unit[(1, bb)][0] for bb in bl if (1, bb) in blk_unit]
        if b_units:
            ui = max(b_units)
            for bb in bl:
                if (1, bb) in blk_unit:
                    uu, off = blk_unit[(1, bb)]
                    consume[ui]["reduces"].append(
                        (bb, uu, off, off + int(S2[bb]),
                         0 if first_w[bb] == 1 else 1))
            for bb in bl:
                if int(S2[bb]) > 0:
                    consume[ui]["epilogue"].append(bb)
        elif not w1_units:
            # group with zero edges anywhere (extremely unlikely)
            consume[0]["epilogue"].extend(bl)
    # h1-exchange point: after the unit where the last h1 block is epilogued
    ep_unit = {}
    for ui, cns in enumerate(consume):
        for bb in cns["epilogue"]:
            ep_unit[bb] = ui
    assert len(ep_unit) == NBLK
    # emit the h1 exchange one unit after the last h1 epilogue so the
    # Pool-issued collective's input wait is nearly satisfied on dispatch
    h1_done = min(max(ep_unit[bb] for bb in range(cfg.NBLK_H1)) + 1,
                  len(units) - 2)
    consume[h1_done]["exch"] = consume[h1_done]["exch"] or []
    consume[h1_done]["exch"].append("h1")
    last_u = len(units) - 1
    consume[last_u]["exch"] = consume[last_u]["exch"] or []
    consume[last_u]["exch"].append("h2")

    # ---- dense per-core arrays ----
    x_perm = np.zeros((NPAD, cfg.F), np.float32)
    x_perm[rmask] = x[pos2old[rmask]]
    dinv_perm = np.ones(NPAD, np.float32)
    dinv_perm[rmask] = dinv[pos2old[rmask]].astype(np.float32)
    bias_perm = np.zeros((NPAD, 16), np.float32)
    bias_perm[rmask] = bias[pos2old[rmask]].astype(np.float32)

    # T3 table + stage (host computed, f64)
    t3_full = np.zeros((NPAD, 16))
    xi = np.zeros((NPAD, cfg.F))
    xi[rmask] = x[pos2old[rmask]].astype(np.float64)
    dv = np.ones(NPAD)
    dv[rmask] = dinv[pos2old[rmask]]
    t3_full = (xi @ M4) * dv[:, None]
    t3_full[~rmask] = 0.0
    import ml_dtypes
    t3tab = np.zeros((NPAD, 128), ml_dtypes.bfloat16)
    t3tab[:, 0:16] = t3_full.astype(ml_dtypes.bfloat16)

    # per-core views: core k's rows at positions pos(k, b, j)
    karr = np.arange(NC)[:, None, None]
    barr = np.arange(NBLK)[None, :, None]
    jarr = np.arange(P)[None, None, :]
    posk = _global_pos(cfg, karr, barr, jarr)        # [NC, NBLK, P]

    xT = [np.ascontiguousarray(
        x_perm[posk[k].reshape(-1)].T) for k in range(NC)]   # [64, PER] block-major
    dinv_blk = [np.ascontiguousarray(dinv_perm[posk[k]].transpose(1, 0))
                for k in range(NC)]                          # [P, NBLK]
    dinv2_blk = [d * d for d in dinv_blk]
    bias_blk = [np.ascontiguousarray(bias_perm[posk[k]].transpose(1, 0, 2))
                for k in range(NC)]                          # [P, NBLK, 16]
    st3 = [np.ascontiguousarray(
        t3_full[posk[k]].transpose(1, 0, 2).astype(np.float32))
        for k in range(NC)]                                  # [P, NBLK, 16]
    mmats = np.ascontiguousarray(np.concatenate([M3, M2, M1], axis=1).astype(np.float32))

    layout = dict(units=units, consume=consume, groups=groups,
                  S1=S1, S2=S2, S3=S3, idxcols=idxcols, posk=posk)
    in_maps = []
    for k in range(NC):
        in_maps.append(dict(
            idx=np.ascontiguousarray(idx_np[k]),
            xT=xT[k],
            dinv_blk=dinv_blk[k],
            dinv2_blk=dinv2_blk[k],
            bias_blk=bias_blk[k],
            mmats=mmats,
            t3tab=t3tab,
            st3=st3[k],
        ))
    return in_maps, layout, old2new


# --------------------------------------------------------------------------
# numpy emulation of the device algorithm (offline validation)
# --------------------------------------------------------------------------

def _algo_sim(in_maps, layout, cfg: Cfg):
    P, NPAD, NBLK, NC = cfg.P, cfg.NPAD, cfg.NBLK, cfg.NCORES
    units, consume = layout["units"], layout["consume"]
    WIN = cfg.WIN
    mm = in_maps[0]["mmats"]

    sts = [in_maps[k]["st3"].copy() for k in range(NC)]     # [P, NBLK, 16]
    tab = in_maps[0]["t3tab"][:, 0:16].copy()               # [NPAD, 16]

    for p in range(4):
        new_sts = [np.zeros((P, NBLK, 16), np.float32) for _ in range(NC)]
        for k in range(NC):
            idx = in_maps[k]["idx"]
            db = in_maps[k]["dinv_blk"]
            d2 = in_maps[k]["dinv2_blk"]
            xTk = in_maps[k]["xT"]
            acc = np.zeros((P, NBLK, 16), np.float32)
            gts = {}
            for ui, u in enumerate(units):
                w = u["w"]
                ws, we = WIN[w]
                nt = u["nslots"] * P
                cols = idx[0:16, u["colbase"]:u["colbase"] + nt // 16]
                flat = cols.T.reshape(-1).astype(np.int64)   # pos = col*16+row
                gts[ui] = tab[ws:we][flat].reshape(u["nslots"], P, 16)
                for (bb, uu, lo, hi, mode) in consume[ui]["reduces"]:
                    r = gts[uu][lo:hi].sum(axis=0)           # [P, 16]
                    if mode == 0:
                        acc[:, bb, :] = r
                    else:
                        acc[:, bb, :] += r
                for bb in consume[ui]["epilogue"]:
                    has_acc = (layout["S1"][bb] + layout["S2"][bb]) > 0
                    t1 = (acc[:, bb, :] if has_acc else 0) + sts[k][:, bb, :]
                    if p < 3:
                        xb = xTk[:, bb * P:(bb + 1) * P].T
                        ps = xb @ mm[:, 16 * p:16 * p + 16]
                        new_sts[k][:, bb, :] = (ps * db[:, bb:bb + 1]
                                                + t1 * d2[:, bb:bb + 1])
                    else:
                        new_sts[k][:, bb, :] = (t1 * db[:, bb:bb + 1]
                                                + in_maps[k]["bias_blk"][:, bb, :])
        # exchange
        if p < 3:
            newtab = np.zeros((NPAD, 16), np.float32)
            posk = layout["posk"]
            for k in range(NC):
                newtab[posk[k].reshape(-1)] = (
                    new_sts[k].transpose(1, 0, 2).reshape(cfg.PER, 16))
            tab = newtab
        sts = new_sts
    return sts


# --------------------------------------------------------------------------
# device module
# --------------------------------------------------------------------------

def _build_module(cfg: Cfg, layout):
    P, PER, NPAD, NBLK, NC = cfg.P, cfg.PER, cfg.NPAD, cfg.NBLK, cfg.NCORES
    units, consume = layout["units"], layout["consume"]
    S1, S2, S3 = layout["S1"], layout["S2"], layout["S3"]
    idxcols = layout["idxcols"]
    WIN = cfg.WIN
    NB1 = cfg.NBLK_H1
    NB2 = NBLK - NB1
    r1, r2 = NB1 * P, NB2 * P

    nc = bacc.Bacc("TRN2", target_bir_lowering=False, debug=False, num_devices=NC,
                   num_swdge_queues=cfg.NQ, dynamic_dma_scratch_size=40960)

    idx = nc.dram_tensor("idx", [128, idxcols], I16, kind="ExternalInput").ap()
    xT = nc.dram_tensor("xT", [cfg.F, PER], F32, kind="ExternalInput").ap()
    dinv_blk = nc.dram_tensor("dinv_blk", [P, NBLK], F32, kind="ExternalInput").ap()
    dinv2_blk = nc.dram_tensor("dinv2_blk", [P, NBLK], F32, kind="ExternalInput").ap()
    bias_blk = nc.dram_tensor("bias_blk", [P, NBLK, 16], F32, kind="ExternalInput").ap()
    mmats = nc.dram_tensor("mmats", [cfg.F, 48], F32, kind="ExternalInput").ap()
    t3tab = nc.dram_tensor("t3tab", [NPAD, 128], BF16, kind="ExternalInput").ap()
    st3 = nc.dram_tensor("st3", [P, NBLK, 16], F32, kind="ExternalInput").ap()
    out = nc.dram_tensor("out", [P, NBLK, 16], F32, kind="ExternalOutput").ap()

    max_sl = max(u["nslots"] for u in units)

    with tile.TileContext(nc) as tc:
        with (
            tc.tile_pool(name="const", bufs=1) as cp,
            tc.tile_pool(name="dram", bufs=1, space="DRAM") as dp,
        ):
            idx_sb = cp.tile([128, idxcols], I16)
            nc.sync.dma_start(idx_sb[:], idx)
            xT_sb = cp.tile([cfg.F, PER], F32)
            nc.sync.dma_start(xT_sb[:], xT)
            mm_sb = cp.tile([cfg.F, 48], F32)
            nc.sync.dma_start(mm_sb[:], mmats)
            db_sb = cp.tile([P, NBLK], F32)
            nc.sync.dma_start(db_sb[:], dinv_blk)
            d2_sb = cp.tile([P, NBLK], F32)
            nc.sync.dma_start(d2_sb[:], dinv2_blk)
            bias_sb = cp.tile([P, NBLK, 16], F32)
            nc.sync.dma_start(bias_sb[:], bias_blk)
            st3_sb = cp.tile([P, NBLK, 16], F32)
            nc.sync.dma_start(st3_sb[:], st3)

            dtabs = [dp.tile([NPAD, 128], BF16, name=f"dtab{i}") for i in range(2)]
            tabs = [t3tab, dtabs[0][:], dtabs[1][:], dtabs[0][:]]
            ccin = [[dp.tile([r1 if h == 0 else r2, 16], BF16,
                             name=f"ccin{e}_{h}") for h in range(2)]
                    for e in range(3)]
            ccout = [[dp.tile([NC * (r1 if h == 0 else r2), 16], BF16,
                              addr_space="Shared", name=f"ccout{e}_{h}")
                      for h in range(2)]
                     for e in range(3)]

            with (
                tc.tile_pool(name="gath", bufs=cfg.GT_BUFS) as gp,
                tc.tile_pool(name="work", bufs=4) as wp,
                tc.tile_pool(name="accp", bufs=2) as ap_,
                tc.tile_pool(name="stage", bufs=2) as sp,
                tc.tile_pool(name="psum", bufs=4, space="PSUM") as psp,
            ):
                chain_tail = [None]

                def chain(binst):
                    if chain_tail[0] is not None:
                        deps = bass.InstructionNameOrderedSet()
                        deps.add(chain_tail[0])
                        binst.ins.add_nosync_dependencies_from(deps)
                    chain_tail[0] = binst.ins.name
                    return binst

                def exchange(e, half, stb_half, target_tab):
                    # e: exchange index 0..2 writes table for pass e+1
                    ci = ccin[e][half]
                    co = ccout[e][half]
                    n_b = NB1 if half == 0 else NB2
                    hh = n_b // 2
                    nc.scalar.dma_start(
                        ci[0:hh * P, :].rearrange("(b p) f -> p b f", p=P),
                        stb_half[:, 0:hh, :])
                    nc.scalar.dma_start(
                        ci[hh * P:, :].rearrange("(b p) f -> p b f", p=P),
                        stb_half[:, hh:, :])
                    nc.gpsimd.collective_compute(
                        "AllGather", mybir.AluOpType.bypass,
                        replica_groups=[list(range(NC))],
                        ins=[ci[:]], outs=[co[:]],
                    )
                    # half-major layout: ccout rows ARE global table rows
                    # (restrides stay on sync only: a scalar-queue restride
                    # would serialize the next ccin DMA behind CC completion)
                    t0 = 0 if half == 0 else cfg.H1ROWS
                    nrows = NC * n_b * P
                    nc.sync.dma_start(
                        target_tab[t0:t0 + nrows, 0:16], co[:])

                qctr = [0]
                st_prev = st3_sb
                st_cur = None
                acc = None

                for p in range(4):
                    st_cur1 = sp.tile([P, NB1, 16], F32, tag="st1")
                    st_cur2 = sp.tile([P, NB2, 16], F32, tag="st2")
                    if p < 3:
                        stb1 = sp.tile([P, NB1, 16], BF16, tag="stb1")
                        stb2 = sp.tile([P, NB2, 16], BF16, tag="stb2")
                    acc = ap_.tile([P, NBLK, 16], F32, tag="acc")
                    gts = {}
                    tab = tabs[p]

                    def stc(b):
                        return (st_cur1[:, b, :] if b < NB1
                                else st_cur2[:, b - NB1, :])

                    def stbc(b):
                        return (stb1[:, b, :] if b < NB1
                                else stb2[:, b - NB1, :])

                    def stp(b):
                        if p == 0:
                            return st_prev[:, b, :]
                        return (st_prev[0][:, b, :] if b < NB1
                                else st_prev[1][:, b - NB1, :])

                    for ui, u in enumerate(units):
                        w = u["w"]
                        ws, we = WIN[w]
                        gt = gp.tile([P, u["nslots"], 128], BF16, tag="gt")
                        gts[ui] = gt
                        n_idx = u["nslots"] * P
                        q = qctr[0] % cfg.NQ
                        qctr[0] += 1
                        chain(nc.gpsimd.dma_gather(
                            out_ap=gt[:],
                            in_ap=tab[ws:we, :],
                            idxs_ap=idx_sb[:, u["colbase"]:
                                           u["colbase"] + n_idx // 16],
                            num_idxs=n_idx,
                            num_idxs_reg=n_idx,
                            elem_size=128,
                            single_packet=cfg.SINGLE_PACKET,
                            prepare_only=False,
                            queue_num=q,
                        ))
                        cns = consume[ui]
                        for (bb, uu, lo, hi, mode) in cns["reduces"]:
                            gsrc = gts[uu]
                            if mode == 0:
                                nc.vector.reduce_sum(
                                    out=acc[:, bb, :],
                                    in_=gsrc[:, lo:hi, 0:16].rearrange(
                                        "p s f -> p f s"),
                                    axis=mybir.AxisListType.X,
                                )
                            else:
                                tmp = wp.tile([P, 16], F32, tag="tmp")
                                nc.vector.reduce_sum(
                                    out=tmp[:],
                                    in_=gsrc[:, lo:hi, 0:16].rearrange(
                                        "p s f -> p f s"),
                                    axis=mybir.AxisListType.X,
                                )
                                nc.vector.tensor_add(
                                    out=acc[:, bb, :], in0=acc[:, bb, :],
                                    in1=tmp[:])
                        for bb in cns["epilogue"]:
                            has_acc = int(S1[bb] + S2[bb] + S3[bb]) > 0
                            t1 = wp.tile([P, 16], F32, tag="t1")
                            if has_acc:
                                nc.vector.tensor_add(out=t1[:],
                                                     in0=acc[:, bb, :],
                                                     in1=stp(bb))
                            else:
                                nc.vector.tensor_copy(out=t1[:], in_=stp(bb))
                            if p < 3:
                                ps = psp.tile([P, 16], F32, tag="ps")
                                nc.tensor.matmul(
                                    out=ps[:],
                                    lhsT=xT_sb[:, bb * P:(bb + 1) * P],
                                    rhs=mm_sb[:, 16 * p:16 * p + 16],
                                    start=True, stop=True)
                                ta = wp.tile([P, 16], F32, tag="ta")
                                nc.scalar.activation(ta[:], ps[:], AF.Copy,
                                                     scale=db_sb[:, bb:bb + 1])
                                tb = wp.tile([P, 16], F32, tag="tb")
                                nc.scalar.activation(tb[:], t1[:], AF.Copy,
                                                     scale=d2_sb[:, bb:bb + 1])
                                nc.vector.tensor_add(out=stc(bb), in0=ta[:],
                                                     in1=tb[:])
                                # bf16 copy feeds the exchange without a bulk
                                # cast on the critical path
                                nc.scalar.activation(stbc(bb), stc(bb), AF.Copy)
                            else:
                                tb = wp.tile([P, 16], F32, tag="tb")
                                nc.scalar.activation(tb[:], t1[:], AF.Copy,
                                                     scale=db_sb[:, bb:bb + 1])
                                nc.vector.tensor_add(out=stc(bb), in0=tb[:],
                                                     in1=bias_sb[:, bb, :])
                        ex = cns["exch"] or []
                        if "h1" in ex and p < 3:
                            exchange(p, 0, stb1, tabs[p + 1])
                        if "h2" in ex:
                            if p < 3:
                                exchange(p, 1, stb2, tabs[p + 1])
                            else:
                                nc.sync.dma_start(out[:, 0:NB1, :], st_cur1[:])
                                nc.sync.dma_start(out[:, NB1:NBLK, :],
                                                  st_cur2[:])
                    st_prev = (st_cur1, st_cur2)

    return nc


# --------------------------------------------------------------------------
# entry point
# --------------------------------------------------------------------------

def _run(inputs, cfg: Cfg, runner=None, **run_kwargs):
    global LAST_RESULTS
    in_maps, layout, old2new = _host_prep(inputs, cfg)
    nc = _build_module(cfg, layout)
    nc.compile()
    if runner is None:
        res = run_bass_kernel_spmd(nc, in_maps, core_ids=list(range(cfg.NCORES)),
                                   **run_kwargs)
        LAST_RESULTS = res
        outs = res.results
    else:
        outs = runner(nc, in_maps)
    full = np.empty((cfg.NPAD, 16), np.float32)
    posk = layout["posk"]
    for k in range(cfg.NCORES):
        o = np.asarray(outs[k]["out"])  # [P, NBLK, 16]
        full[posk[k].reshape(-1)] = o.transpose(1, 0, 2).reshape(cfg.PER, 16)
    return full[old2new]


def kernel(**inputs) -> np.ndarray:
    return _run(inputs, CFG)


# revision 19
# speedup vs baseline: 1.0406x; 1.0406x over previous
"""Trainium2 Bass kernel v2: DGCNN forward (4-layer GCN + Conv1d readout) on 8 cores.

Math (same as v1): with A = D^-1/2 (Adj + I) D^-1/2,
    out = A(x M1 + A(x M2 + A(x M3 + A(x M4)))) + bias-table
All 4 aggregation passes are width-16 gathers + segment sums.

v2 structural changes vs v1 (v1 = kernel_baseline.py):
  - Half-major global node layout: position = h1: k*4096 + b*128 + j (blocks
    0..31), h2: 32768 + k*2176 + (b-32)*128 + j (blocks 32..48). The half-1
    exchange output IS table rows [0, 32768) = gather window W1, so next-pass
    W1 gathers only wait on the half-1 collective (emitted mid-pass) and the
    pass tail overlaps the half-2 exchange.
  - Two gather windows W1=[0,32768), W2=[17408,50176); flex edges (src in the
    overlap) prefer W1 so the early-firing W1 phase carries ~50% of the work.
  - Pass-0 table (T3 = dinv*(x@M4)) and stage are host-precomputed inputs, so
    pass-0 gathers start right after the idx tensor loads (no startup
    exchange).
  - 48 gather units per pass (16 groups x {W1a, W1b, W2}) rotating across
    the 4 SWDGE queues = 4 Q7 desc-gen core pairs; deep gt pools keep all 4
    pairs busy (desc-gen is the machine bottleneck at ~8ns/desc/pair).
  - Iterated (deg, f2) node dealing cuts ELL padding 1.48x -> 1.27x; bf16
    message tables halve the exchange payload; h1-group units are emitted
    first so the half-1 collective dispatches at ~60% of the pass stream.
  - Per-block accumulator tiles decouple W1-phase and W2-phase consumption.
"""

import dataclasses
import numpy as np

import concourse.bass as bass
import concourse.bacc as bacc
import concourse.tile as tile
from concourse import mybir
from concourse.bass_utils import run_bass_kernel_spmd

F32 = mybir.dt.float32
BF16 = mybir.dt.bfloat16
I16 = mybir.dt.int16
AF = mybir.ActivationFunctionType


@dataclasses.dataclass(frozen=True)
class Cfg:
    N: int = 50000
    F: int = 64
    NCORES: int = 8
    P: int = 128
    NBLK: int = 49
    NBLK_H1: int = 32          # h1 blocks; h1 rows = 8*32*128 = 32768 = W1
    NGRP_H1: int = 10
    NGRP_H2: int = 6
    NQ: int = 4
    SINGLE_PACKET: bool = False
    GT_BUFS: int = 9
    W1_PRE: int = 7            # W1 groups emitted before the h1 W2 block

    @property
    def PER(self):
        return self.NBLK * self.P          # 6272 rows per core

    @property
    def NPAD(self):
        return self.NCORES * self.PER      # 50176

    @property
    def H1ROWS(self):
        return self.NCORES * self.NBLK_H1 * self.P   # 32768

    @property
    def NGRP(self):
        return self.NGRP_H1 + self.NGRP_H2

    # window [start, end) in global positions
    @property
    def WIN(self):
        return ((0, 32768), (self.NPAD - 32768, self.NPAD))


CFG = Cfg()

LAST_RESULTS = None


# --------------------------------------------------------------------------
# host preprocessing
# --------------------------------------------------------------------------

def _global_pos(cfg, k, b, j):
    b = np.asarray(b)
    h1 = b < cfg.NBLK_H1
    r1 = cfg.NBLK_H1 * cfg.P
    r2 = (cfg.NBLK - cfg.NBLK_H1) * cfg.P
    return np.where(
        h1,
        k * r1 + b * cfg.P + j,
        cfg.H1ROWS + k * r2 + (b - cfg.NBLK_H1) * cfg.P + j,
    )


def _host_prep(inputs, cfg: Cfg):
    x = np.asarray(inputs["x"], np.float32)
    ei = np.asarray(inputs["edge_index"]).astype(np.int64)
    W = [np.asarray(inputs[f"W{i}"], np.float64) for i in range(4)]
    b = [np.asarray(inputs[f"b{i}"], np.float64) for i in range(4)]
    conv_w = np.asarray(inputs["conv_w"], np.float64)
    conv_b = np.asarray(inputs["conv_b"], np.float64)

    n = x.shape[0]
    assert n == cfg.N and x.shape[1] == cfg.F
    P, PER, NPAD, NBLK, NC = cfg.P, cfg.PER, cfg.NPAD, cfg.NBLK, cfg.NCORES

    src_l = np.concatenate([ei[0], np.arange(n, dtype=np.int64)])
    dst_l = np.concatenate([ei[1], np.arange(n, dtype=np.int64)])
    deg = np.bincount(dst_l, minlength=n).astype(np.float64)
    dinv = 1.0 / np.sqrt(np.maximum(deg, 1.0))

    # ---- weight-derived small matrices ----
    Cw = [conv_w[:, 0:64], conv_w[:, 64:128], conv_w[:, 128:192], conv_w[:, 192:193]]
    M1 = W[0] @ Cw[0].T
    M2 = W[0] @ W[1] @ Cw[1].T
    M3 = W[0] @ W[1] @ W[2] @ Cw[2].T
    M4 = W[0] @ W[1] @ W[2] @ W[3] @ Cw[3].T
    c0 = b[0] @ Cw[0].T + b[1] @ Cw[1].T + b[2] @ Cw[2].T + b[3] @ Cw[3].T + conv_b
    c1 = (b[0] @ W[1]) @ Cw[1].T + (b[1] @ W[2]) @ Cw[2].T + (b[2] @ W[3]) @ Cw[3].T
    c2 = (b[0] @ W[1] @ W[2]) @ Cw[2].T + (b[1] @ W[2] @ W[3]) @ Cw[3].T
    c3 = (b[0] @ W[1] @ W[2] @ W[3]) @ Cw[3].T

    def aggv(v):
        o = np.zeros(n)
        np.add.at(o, dst_l, (v * dinv)[src_l])
        return o * dinv

    v1 = aggv(np.ones(n))
    v2 = aggv(v1)
    v3 = aggv(v2)
    bias = (np.outer(np.ones(n), c0) + np.outer(v1, c1)
            + np.outer(v2, c2) + np.outer(v3, c3))  # [n, 16]

    # ---- permutation: iterated (deg, f2) dealing into (class, core, j) ----
    # Sorting nodes by (deg desc, f2 asc) before dealing equalizes both the
    # block degree max AND the forced-W2 max within each block, cutting ELL
    # padding ~1.48x -> ~1.25x.  f2 depends on src positions, so iterate.
    ndeg_n = (deg - 1).astype(np.int64)     # non-self in-degree

    def build_perm(order):
        order_p = np.concatenate([order, np.full(NPAD - n, -1, np.int64)])
        deg_p = np.zeros(NPAD)
        real_rank = order_p >= 0
        deg_p[real_rank] = deg[order_p[real_rank]] - 1.0
        cls_of_rank = np.arange(NPAD) // P // NC
        mTc = np.zeros(NBLK)
        np.maximum.at(mTc, cls_of_rank, deg_p)
        # group classes; h1 groups draw from classes 0..31, h2 from 32..48
        cap = ([4, 4] + [3] * (cfg.NGRP_H1 - 2)) + ([3] * (cfg.NGRP_H2 - 1) + [2])
        assert sum(cap[:cfg.NGRP_H1]) == cfg.NBLK_H1 and sum(cap) == NBLK
        groups_c = [[] for _ in range(cfg.NGRP)]
        gsum = np.zeros(cfg.NGRP)
        for bq in np.argsort(-mTc, kind="stable"):
            lo, hi = (0, cfg.NGRP_H1) if bq < cfg.NBLK_H1 else (cfg.NGRP_H1, cfg.NGRP)
            cand = sorted(range(lo, hi),
                          key=lambda q: (len(groups_c[q]) >= cap[q], gsum[q], q))
            q = cand[0]
            groups_c[q].append(int(bq))
            gsum[q] += mTc[bq]
        order_cls = [c for q in range(cfg.NGRP) for c in groups_c[q]]
        renum = np.zeros(NBLK, np.int64)
        for newid, c in enumerate(order_cls):
            renum[c] = newid
        groups = []
        pos = 0
        for q in range(cfg.NGRP):
            groups.append(list(range(pos, pos + len(groups_c[q]))))
            pos += len(groups_c[q])
        g = np.arange(NPAD) // P
        j = np.arange(NPAD) % P
        npos_of_rank = _global_pos(cfg, g % NC, renum[g // NC], j)
        return order_p, renum, groups, npos_of_rank

    order = np.argsort(-deg, kind="stable")
    for _ in range(5):
        order_p, renum, groups, npos_of_rank = build_perm(order)
        o2p = np.full(n, -1, np.int64)
        m_ = order_p >= 0
        o2p[order_p[m_]] = npos_of_rank[m_]
        s_pos = o2p[ei[0]]
        f2n = np.bincount(ei[1][s_pos >= 32768], minlength=n)
        order = np.lexsort((f2n, -ndeg_n))
    order_p, renum, groups, npos_of_rank = build_perm(order)

    # force dummy (pad) nodes at positions 32767 (in W1&W3) and NPAD-1 (in W2)
    pos2rank = np.empty(NPAD, np.int64)
    pos2rank[npos_of_rank] = np.arange(NPAD)
    for dpos in (32767, NPAD - 1):
        r_t = pos2rank[dpos]
        if order_p[r_t] >= 0:
            # swap with some rank that is a pad
            pads = np.nonzero(order_p < 0)[0]
            r_p = pads[-1]
            order_p[r_t], order_p[r_p] = order_p[r_p], order_p[r_t]
    Z1, Z2 = 32767, NPAD - 1      # zero rows: Z1 in W1/W3 range, Z2 in W2
    assert order_p[pos2rank[Z1]] < 0 and order_p[pos2rank[Z2]] < 0

    pos2old = np.full(NPAD, -1, np.int64)
    pos2old[npos_of_rank] = order_p
    old2new = np.full(n, -1, np.int64)
    rmask = pos2old >= 0
    old2new[pos2old[rmask]] = np.nonzero(rmask)[0]
    assert (old2new >= 0).all()

    # ---- per-block 2-window split (flex overlap prefers W1) ----
    (w1s, w1e), (w2s, w2e) = cfg.WIN
    s_new = old2new[ei[0]]
    d_new = old2new[ei[1]]

    # dst position -> (core, block, j)
    def pos_to_kbj(pos):
        h1 = pos < cfg.H1ROWS
        r1 = cfg.NBLK_H1 * P
        r2 = (NBLK - cfg.NBLK_H1) * P
        k = np.where(h1, pos // r1, (pos - cfg.H1ROWS) // r2)
        rem = np.where(h1, pos - k * r1, pos - cfg.H1ROWS - k * r2)
        blk = np.where(h1, rem // P, cfg.NBLK_H1 + rem // P)
        jj = rem % P
        return k, blk, jj

    e_k, e_b, e_j = pos_to_kbj(d_new)
    # edge class: 0=F1(W1 only) 1=flex(W1 or W2) 2=F2(W2 only)
    ecls = np.where(s_new < w2s, 0, np.where(s_new < w1e, 1, 2))

    cnt = np.zeros((3, NPAD), np.int64)
    for c in range(3):
        cnt[c] = np.bincount(d_new[ecls == c], minlength=NPAD)
    f1, mfx, f2 = cnt
    degp = f1 + mfx + f2

    blk_of_pos = pos_to_kbj(np.arange(NPAD))[1]
    S1 = np.zeros(NBLK, np.int64)
    S2 = np.zeros(NBLK, np.int64)
    for bq in range(NBLK):
        sel = blk_of_pos == bq
        mT = int(degp[sel].max())
        mf1 = int(f1[sel].max())
        mf2 = int(f2[sel].max())
        tot = max(mT, mf1 + mf2)
        S2[bq] = mf2            # W2 minimal -> W1 (early phase) maximal
        S1[bq] = tot - mf2
    S3 = np.zeros(NBLK, np.int64)   # no third window

    S1p = S1[blk_of_pos]
    S2p = S2[blk_of_pos]
    a_j = np.minimum(mfx, S1p - f1)        # flex edges going to W1
    n1 = f1 + a_j
    n2 = degp - n1
    assert (n1 <= S1p).all() and (n2 <= S2p).all()

    # ---- per-edge window + slot assignment ----
    eo = np.argsort(d_new, kind="stable")
    d_s = d_new[eo]
    s_s = s_new[eo]
    c_s = ecls[eo]
    starts = np.searchsorted(d_s, np.arange(NPAD + 1))

    def rank_within(mask):
        cm = np.concatenate([[0], np.cumsum(mask)])
        return cm[:-1] - cm[starts[d_s]]

    is_fx = c_s == 1
    rfx = rank_within(is_fx)
    goW1 = (c_s == 0) | (is_fx & (rfx < a_j[d_s]))
    win_of = np.where(goW1, 0, 1)
    slot = np.empty(len(d_s), np.int64)
    for w, m in enumerate((goW1, ~goW1)):
        slot[m] = rank_within(m)[m]
    nw = np.stack([n1, n2])
    assert (slot < nw[win_of, d_s]).all()

    # ---- units ----
    # phase A: for g: W1a(g), W1b(g)  (split group blocks ~in half)
    # phase B: for g: W2(g)           (h1 groups first == natural order)
    Sw = [S1, S2]
    units = []          # dicts: w, g, blocks(list), nslots, colbase
    blk_unit = {}       # (w, b) -> (unit_idx, slot_offset_within_unit)
    colbase = 0

    def add_unit(w, gq, blks):
        nonlocal colbase
        ns = int(sum(Sw[w][bb] for bb in blks))
        if ns == 0:
            return
        u = dict(w=w, g=gq, blocks=list(blks), nslots=ns, colbase=colbase)
        off = 0
        for bb in blks:
            blk_unit[(w, bb)] = (len(units), off)
            off += int(Sw[w][bb])
        colbase += ns * P // 16
        units.append(u)

    # h1 groups' W1 AND W2 units first: their epilogues complete ~60% into
    # the stream, so the h1 collective dispatches early and its latency
    # overlaps the h2-group units that follow.
    def add_w1(gq):
        bl = groups[gq]
        h = (len(bl) + 1) // 2
        add_unit(0, gq, bl[:h])
        add_unit(0, gq, bl[h:])

    w1_next = 0
    while w1_next < cfg.W1_PRE:
        add_w1(w1_next)
        w1_next += 1
    for gq in range(cfg.NGRP_H1):
        while w1_next <= gq:          # invariant: W1(g) precedes W2(g)
            add_w1(w1_next)
            w1_next += 1
        add_unit(1, gq, groups[gq])
        if gq % 2 == 0 and w1_next < cfg.NGRP_H1:
            add_w1(w1_next)
            w1_next += 1
    while w1_next < cfg.NGRP:
        add_w1(w1_next)
        w1_next += 1
    for gq in range(cfg.NGRP_H1, cfg.NGRP):
        add_unit(1, gq, groups[gq])
    idxcols = colbase

    # ---- idx tensor build ----
    WSTART = np.array([w1s, w2s], np.int64)
    ZPAD = np.array([Z1 - w1s, Z2 - w2s], np.int64)
    idx_np = np.empty((NC, 128, idxcols), np.int16)
    for u in units:
        c0_, c1_ = u["colbase"], u["colbase"] + u["nslots"] * P // 16
        idx_np[:, :, c0_:c1_] = np.int16(ZPAD[u["w"]])

    e_uid = np.empty(len(d_s), np.int64)
    e_uoff = np.empty(len(d_s), np.int64)
    for (w, bb), (ui, off) in blk_unit.items():
        m = (win_of == w) & (e_b[eo] == bb)
        e_uid[m] = ui
        e_uoff[m] = off
    ub = np.array([u["colbase"] for u in units], np.int64)
    posn = (e_uoff + slot) * P + e_j[eo]
    col = ub[e_uid] + posn // 16
    row = posn % 16
    val = (s_s - WSTART[win_of]).astype(np.int16)
    assert (val >= 0).all()
    ek = e_k[eo]
    for k in range(NC):
        mk = ek == k
        for r in range(8):
            idx_np[k, row[mk] + 16 * r, col[mk]] = val[mk]

    # ---- consumption plan (host-side schedule metadata) ----
    # For each unit index: list of (block, src_unit, lo, hi, mode) reduces
    # after this unit, mode: 0 = write acc, 1 = add to acc.  Plus epilogues.
    first_w = np.where(S1 > 0, 0, 1)     # first window with slots
    consume = [dict(reduces=[], epilogue=[], exch=None) for _ in units]
    for gq in range(cfg.NGRP):
        bl = groups[gq]
        # phase A consumption: after the last W1 unit of this group
        w1_units = [blk_unit[(0, bb)][0] for bb in bl if (0, bb) in blk_unit]
        if w1_units:
            ui = max(w1_units)
            for bb in bl:
                if (0, bb) in blk_unit:
                    uu, off = blk_unit[(0, bb)]
                    consume[ui]["reduces"].append(
                        (bb, uu, off, off + int(S1[bb]), 0))
            for bb in bl:
                if int(S2[bb]) == 0:
                    consume[ui]["epilogue"].append(bb)   # W1-only blocks
        # phase B: after W2(g)
        b_units = [blk_unit[(1, bb)][0] for bb in bl if (1, bb) in blk_unit]
        if b_units:
            ui = max(b_units)
            for bb in bl:
                if (1, bb) in blk_unit:
                    uu, off = blk_unit[(1, bb)]
                    consume[ui]["reduces"].append(
                        (bb, uu, off, off + int(S2[bb]),
                         0 if first_w[bb] == 1 else 1))
            for bb in bl:
                if int(S2[bb]) > 0:
                    consume[ui]["epilogue"].append(bb)
        elif not w1_units:
            # group with zero edges anywhere (extremely unlikely)
            consume[0]["epilogue"].extend(bl)
    # h1-exchange point: after the unit where the last h1 block is epilogued
    ep_unit = {}
    for ui, cns in enumerate(consume):
        for bb in cns["epilogue"]:
            ep_unit[bb] = ui
    assert len(ep_unit) == NBLK
    # emit the h1 exchange one unit after the last h1 epilogue so the
    # Pool-issued collective's input wait is nearly satisfied on dispatch
    h1_done = min(max(ep_unit[bb] for bb in range(cfg.NBLK_H1)) + 1,
                  len(units) - 2)
    consume[h1_done]["exch"] = consume[h1_done]["exch"] or []
    consume[h1_done]["exch"].append("h1")
    last_u = len(units) - 1
    consume[last_u]["exch"] = consume[last_u]["exch"] or []
    consume[last_u]["exch"].append("h2")

    # ---- dense per-core arrays ----
    x_perm = np.zeros((NPAD, cfg.F), np.float32)
    x_perm[rmask] = x[pos2old[rmask]]
    dinv_perm = np.ones(NPAD, np.float32)
    dinv_perm[rmask] = dinv[pos2old[rmask]].astype(np.float32)
    bias_perm = np.zeros((NPAD, 16), np.float32)
    bias_perm[rmask] = bias[pos2old[rmask]].astype(np.float32)

    # T3 table + stage (host computed, f64)
    t3_full = np.zeros((NPAD, 16))
    xi = np.zeros((NPAD, cfg.F))
    xi[rmask] = x[pos2old[rmask]].astype(np.float64)
    dv = np.ones(NPAD)
    dv[rmask] = dinv[pos2old[rmask]]
    t3_full = (xi @ M4) * dv[:, None]
    t3_full[~rmask] = 0.0
    import ml_dtypes
    t3tab = np.zeros((NPAD, 128), ml_dtypes.bfloat16)
    t3tab[:, 0:16] = t3_full.astype(ml_dtypes.bfloat16)

    # per-core views: core k's rows at positions pos(k, b, j)
    karr = np.arange(NC)[:, None, None]
    barr = np.arange(NBLK)[None, :, None]
    jarr = np.arange(P)[None, None, :]
    posk = _global_pos(cfg, karr, barr, jarr)        # [NC, NBLK, P]

    xT = [np.ascontiguousarray(
        x_perm[posk[k].reshape(-1)].T) for k in range(NC)]   # [64, PER] block-major
    dinv_blk = [np.ascontiguousarray(dinv_perm[posk[k]].transpose(1, 0))
                for k in range(NC)]                          # [P, NBLK]
    dinv2_blk = [d * d for d in dinv_blk]
    bias_blk = [np.ascontiguousarray(bias_perm[posk[k]].transpose(1, 0, 2))
                for k in range(NC)]                          # [P, NBLK, 16]
    st3 = [np.ascontiguousarray(
        t3_full[posk[k]].transpose(1, 0, 2).astype(np.float32))
        for k in range(NC)]                                  # [P, NBLK, 16]
    mmats = np.ascontiguousarray(np.concatenate([M3, M2, M1], axis=1).astype(np.float32))

    layout = dict(units=units, consume=consume, groups=groups,
                  S1=S1, S2=S2, S3=S3, idxcols=idxcols, posk=posk)
    in_maps = []
    for k in range(NC):
        in_maps.append(dict(
            idx=np.ascontiguousarray(idx_np[k]),
            xT=xT[k],
            dinv_blk=dinv_blk[k],
            dinv2_blk=dinv2_blk[k],
            bias_blk=bias_blk[k],
            mmats=mmats,
            t3tab=t3tab,
            st3=st3[k],
        ))
    return in_maps, layout, old2new


# --------------------------------------------------------------------------
# numpy emulation of the device algorithm (offline validation)
# --------------------------------------------------------------------------

def _algo_sim(in_maps, layout, cfg: Cfg):
    P, NPAD, NBLK, NC = cfg.P, cfg.NPAD, cfg.NBLK, cfg.NCORES
    units, consume = layout["units"], layout["consume"]
    WIN = cfg.WIN
    mm = in_maps[0]["mmats"]

    sts = [in_maps[k]["st3"].copy() for k in range(NC)]     # [P, NBLK, 16]
    tab = in_maps[0]["t3tab"][:, 0:16].copy()               # [NPAD, 16]

    for p in range(4):
        new_sts = [np.zeros((P, NBLK, 16), np.float32) for _ in range(NC)]
        for k in range(NC):
            idx = in_maps[k]["idx"]
            db = in_maps[k]["dinv_blk"]
            d2 = in_maps[k]["dinv2_blk"]
            xTk = in_maps[k]["xT"]
            acc = np.zeros((P, NBLK, 16), np.float32)
            gts = {}
            for ui, u in enumerate(units):
                w = u["w"]
                ws, we = WIN[w]
                nt = u["nslots"] * P
                cols = idx[0:16, u["colbase"]:u["colbase"] + nt // 16]
                flat = cols.T.reshape(-1).astype(np.int64)   # pos = col*16+row
                gts[ui] = tab[ws:we][flat].reshape(u["nslots"], P, 16)
                for (bb, uu, lo, hi, mode) in consume[ui]["reduces"]:
                    r = gts[uu][lo:hi].sum(axis=0)           # [P, 16]
                    if mode == 0:
                        acc[:, bb, :] = r
                    else:
                        acc[:, bb, :] += r
                for bb in consume[ui]["epilogue"]:
                    has_acc = (layout["S1"][bb] + layout["S2"][bb]) > 0
                    t1 = (acc[:, bb, :] if has_acc else 0) + sts[k][:, bb, :]
                    if p < 3:
                        xb = xTk[:, bb * P:(bb + 1) * P].T
                        ps = xb @ mm[:, 16 * p:16 * p + 16]
                        new_sts[k][:, bb, :] = (ps * db[:, bb:bb + 1]
                                                + t1 * d2[:, bb:bb + 1])
                    else:
                        new_sts[k][:, bb, :] = (t1 * db[:, bb:bb + 1]
                                                + in_maps[k]["bias_blk"][:, bb, :])
        # exchange
        if p < 3:
            newtab = np.zeros((NPAD, 16), np.float32)
            posk = layout["posk"]
            for k in range(NC):
                newtab[posk[k].reshape(-1)] = (
                    new_sts[k].transpose(1, 0, 2).reshape(cfg.PER, 16))
            tab = newtab
        sts = new_sts
    return sts


# --------------------------------------------------------------------------
# device module
# --------------------------------------------------------------------------

def _build_module(cfg: Cfg, layout):
    P, PER, NPAD, NBLK, NC = cfg.P, cfg.PER, cfg.NPAD, cfg.NBLK, cfg.NCORES
    units, consume = layout["units"], layout["consume"]
    S1, S2, S3 = layout["S1"], layout["S2"], layout["S3"]
    idxcols = layout["idxcols"]
    WIN = cfg.WIN
    NB1 = cfg.NBLK_H1
    NB2 = NBLK - NB1
    r1, r2 = NB1 * P, NB2 * P

    nc = bacc.Bacc("TRN2", target_bir_lowering=False, debug=False, num_devices=NC,
                   num_swdge_queues=cfg.NQ, dynamic_dma_scratch_size=40960)

    idx = nc.dram_tensor("idx", [128, idxcols], I16, kind="ExternalInput").ap()
    xT = nc.dram_tensor("xT", [cfg.F, PER], F32, kind="ExternalInput").ap()
    dinv_blk = nc.dram_tensor("dinv_blk", [P, NBLK], F32, kind="ExternalInput").ap()
    dinv2_blk = nc.dram_tensor("dinv2_blk", [P, NBLK], F32, kind="ExternalInput").ap()
    bias_blk = nc.dram_tensor("bias_blk", [P, NBLK, 16], F32, kind="ExternalInput").ap()
    mmats = nc.dram_tensor("mmats", [cfg.F, 48], F32, kind="ExternalInput").ap()
    t3tab = nc.dram_tensor("t3tab", [NPAD, 128], BF16, kind="ExternalInput").ap()
    st3 = nc.dram_tensor("st3", [P, NBLK, 16], F32, kind="ExternalInput").ap()
    out = nc.dram_tensor("out", [P, NBLK, 16], F32, kind="ExternalOutput").ap()

    max_sl = max(u["nslots"] for u in units)

    with tile.TileContext(nc) as tc:
        with (
            tc.tile_pool(name="const", bufs=1) as cp,
            tc.tile_pool(name="dram", bufs=1, space="DRAM") as dp,
        ):
            idx_sb = cp.tile([128, idxcols], I16)
            nc.sync.dma_start(idx_sb[:], idx)
            xT_sb = cp.tile([cfg.F, PER], F32)
            nc.sync.dma_start(xT_sb[:], xT)
            mm_sb = cp.tile([cfg.F, 48], F32)
            nc.sync.dma_start(mm_sb[:], mmats)
            db_sb = cp.tile([P, NBLK], F32)
            nc.sync.dma_start(db_sb[:], dinv_blk)
            d2_sb = cp.tile([P, NBLK], F32)
            nc.sync.dma_start(d2_sb[:], dinv2_blk)
            bias_sb = cp.tile([P, NBLK, 16], F32)
            nc.sync.dma_start(bias_sb[:], bias_blk)
            st3_sb = cp.tile([P, NBLK, 16], F32)
            nc.sync.dma_start(st3_sb[:], st3)

            dtabs = [dp.tile([NPAD, 128], BF16, name=f"dtab{i}") for i in range(2)]
            tabs = [t3tab, dtabs[0][:], dtabs[1][:], dtabs[0][:]]
            ccin = [[dp.tile([r1 if h == 0 else r2, 16], BF16,
                             name=f"ccin{e}_{h}") for h in range(2)]
                    for e in range(3)]
            ccout = [[dp.tile([NC * (r1 if h == 0 else r2), 16], BF16,
                              addr_space="Shared", name=f"ccout{e}_{h}")
                      for h in range(2)]
                     for e in range(3)]

            with (
                tc.tile_pool(name="gath", bufs=cfg.GT_BUFS) as gp,
                tc.tile_pool(name="work", bufs=6) as wp,
                tc.tile_pool(name="accp", bufs=2) as ap_,
                tc.tile_pool(name="stage", bufs=2) as sp,
                tc.tile_pool(name="psum", bufs=6, space="PSUM") as psp,
            ):
                chain_tail = [None]

                def chain(binst):
                    if chain_tail[0] is not None:
                        deps = bass.InstructionNameOrderedSet()
                        deps.add(chain_tail[0])
                        binst.ins.add_nosync_dependencies_from(deps)
                    chain_tail[0] = binst.ins.name
                    return binst

                def exchange(e, half, stb_half, target_tab):
                    # e: exchange index 0..2 writes table for pass e+1
                    ci = ccin[e][half]
                    co = ccout[e][half]
                    n_b = NB1 if half == 0 else NB2
                    hh = n_b // 2
                    nc.scalar.dma_start(
                        ci[0:hh * P, :].rearrange("(b p) f -> p b f", p=P),
                        stb_half[:, 0:hh, :])
                    nc.scalar.dma_start(
                        ci[hh * P:, :].rearrange("(b p) f -> p b f", p=P),
                        stb_half[:, hh:, :])
                    nc.gpsimd.collective_compute(
                        "AllGather", mybir.AluOpType.bypass,
                        replica_groups=[list(range(NC))],
                        ins=[ci[:]], outs=[co[:]],
                    )
                    # half-major layout: ccout rows ARE global table rows
                    # (restrides stay on sync only: a scalar-queue restride
                    # would serialize the next ccin DMA behind CC completion)
                    t0 = 0 if half == 0 else cfg.H1ROWS
                    nrows = NC * n_b * P
                    nc.sync.dma_start(
                        target_tab[t0:t0 + nrows, 0:16], co[:])

                qctr = [0]
                st_prev = st3_sb
                st_cur = None
                acc = None

                for p in range(4):
                    st_cur1 = sp.tile([P, NB1, 16], F32, tag="st1")
                    st_cur2 = sp.tile([P, NB2, 16], F32, tag="st2")
                    if p < 3:
                        stb1 = sp.tile([P, NB1, 16], BF16, tag="stb1")
                        stb2 = sp.tile([P, NB2, 16], BF16, tag="stb2")
                    acc = ap_.tile([P, NBLK, 16], F32, tag="acc")
                    gts = {}
                    tab = tabs[p]

                    def stc(b):
                        return (st_cur1[:, b, :] if b < NB1
                                else st_cur2[:, b - NB1, :])

                    def stbc(b):
                        return (stb1[:, b, :] if b < NB1
                                else stb2[:, b - NB1, :])

                    def stp(b):
                        if p == 0:
                            return st_prev[:, b, :]
                        return (st_prev[0][:, b, :] if b < NB1
                                else st_prev[1][:, b - NB1, :])

                    for ui, u in enumerate(units):
                        w = u["w"]
                        ws, we = WIN[w]
                        gt = gp.tile([P, u["nslots"], 128], BF16, tag="gt")
                        gts[ui] = gt
                        n_idx = u["nslots"] * P
                        q = qctr[0] % cfg.NQ
                        qctr[0] += 1
                        chain(nc.gpsimd.dma_gather(
                            out_ap=gt[:],
                            in_ap=tab[ws:we, :],
                            idxs_ap=idx_sb[:, u["colbase"]:
                                           u["colbase"] + n_idx // 16],
                            num_idxs=n_idx,
                            num_idxs_reg=n_idx,
                            elem_size=128,
                            single_packet=cfg.SINGLE_PACKET,
                            prepare_only=False,
                            queue_num=q,
                        ))
                        cns = consume[ui]
                        for (bb, uu, lo, hi, mode) in cns["reduces"]:
                            gsrc = gts[uu]
                            if mode == 0:
                                nc.vector.reduce_sum(
                                    out=acc[:, bb, :],
                                    in_=gsrc[:, lo:hi, 0:16].rearrange(
                                        "p s f -> p f s"),
                                    axis=mybir.AxisListType.X,
                                )
                            else:
                                tmp = wp.tile([P, 16], F32, tag="tmp")
                                nc.vector.reduce_sum(
                                    out=tmp[:],
                                    in_=gsrc[:, lo:hi, 0:16].rearrange(
                                        "p s f -> p f s"),
                                    axis=mybir.AxisListType.X,
                                )
                                nc.vector.tensor_add(
                                    out=acc[:, bb, :], in0=acc[:, bb, :],
                                    in1=tmp[:])
                        for bb in cns["epilogue"]:
                            has_acc = int(S1[bb] + S2[bb] + S3[bb]) > 0
                            t1 = wp.tile([P, 16], F32, tag="t1")
                            if has_acc:
                                nc.vector.tensor_add(out=t1[:],
                                                     in0=acc[:, bb, :],
                                                     in1=stp(bb))
                            else:
                                nc.vector.tensor_copy(out=t1[:], in_=stp(bb))
                            if p < 3:
                                ps = psp.tile([P, 16], F32, tag="ps")
                                nc.tensor.matmul(
                                    out=ps[:],
                                    lhsT=xT_sb[:, bb * P:(bb + 1) * P],
                                    rhs=mm_sb[:, 16 * p:16 * p + 16],
                                    start=True, stop=True)
                                ta = wp.tile([P, 16], F32, tag="ta")
                                nc.scalar.activation(ta[:], ps[:], AF.Copy,
                                                     scale=db_sb[:, bb:bb + 1])
                                tb = wp.tile([P, 16], F32, tag="tb")
                                nc.scalar.activation(tb[:], t1[:], AF.Copy,
                                                     scale=d2_sb[:, bb:bb + 1])
                                nc.vector.tensor_add(out=stc(bb), in0=ta[:],
                                                     in1=tb[:])
                                # bf16 copy feeds the exchange without a bulk
                                # cast on the critical path
                                nc.scalar.activation(stbc(bb), stc(bb), AF.Copy)
                            else:
                                tb = wp.tile([P, 16], F32, tag="tb")
                                nc.scalar.activation(tb[:], t1[:], AF.Copy,
                                                     scale=db_sb[:, bb:bb + 1])
                                nc.vector.tensor_add(out=stc(bb), in0=tb[:],
                                                     in1=bias_sb[:, bb, :])
                        ex = cns["exch"] or []
                        if "h1" in ex and p < 3:
                            exchange(p, 0, stb1, tabs[p + 1])
                        if "h2" in ex:
                            if p < 3:
                                exchange(p, 1, stb2, tabs[p + 1])
                            else:
                                nc.sync.dma_start(out[:, 0:NB1, :], st_cur1[:])
                                nc.sync.dma_start(out[:, NB1:NBLK, :],
                                                  st_cur2[:])
                    st_prev = (st_cur1, st_cur2)

    return nc


# --------------------------------------------------------------------------
# entry point
# --------------------------------------------------------------------------

def _run(inputs, cfg: Cfg, runner=None, **run_kwargs):
    global LAST_RESULTS
    in_maps, layout, old2new = _host_prep(inputs, cfg)
    nc = _build_module(cfg, layout)
    nc.compile()
    if runner is None:
        res = run_bass_kernel_spmd(nc, in_maps, core_ids=list(range(cfg.NCORES)),
                                   **run_kwargs)
        LAST_RESULTS = res
        outs = res.results
    else:
        outs = runner(nc, in_maps)
    full = np.empty((cfg.NPAD, 16), np.float32)
    posk = layout["posk"]
    for k in range(cfg.NCORES):
        o = np.asarray(outs[k]["out"])  # [P, NBLK, 16]
        full[posk[k].reshape(-1)] = o.transpose(1, 0, 2).reshape(cfg.PER, 16)
    return full[old2new]


def kernel(**inputs) -> np.ndarray:
    return _run(inputs, CFG)
